# revision 10
# baseline (speedup 1.0000x reference)
"""Trainium2 Bass kernel for nn_MultiHeadAttention_8572754722984.

Full-input contract: kernel(**inputs) takes the complete tensors
(B=4, S=2048, D=1024, H=16, dk=dv=64) and returns [B, S, 1024] fp32.

Sharding: 8 cores = 4 batches x 2 head-halves. Core c handles batch c//2
and heads (c%2)*8 .. (c%2)*8+8. Everything is independent per core (no
collectives): projections use the per-core weight slice, attention chains
stay within a head.

Matmul operands are fp16 (PE runs fp16 at full rate — 4x faster than fp32
— with fp32 PSUM accumulation; end-to-end relative error ~7e-4). Q/K/V and
the weight slices are cast to fp16 on the host, which also enables the
hardware DMA-transpose (2-byte dtypes only) for loading X^T directly.

Per-core dataflow:
  - DMA-transpose Q/K/V [S,D] fp16 -> X^T [D,S] in SBUF.
  - q^T,k^T per head-pair: [128 (2x64 dk), S] = W_slice^T @ X^T (PSUM-accum
    over D chunks), bias added during the PSUM->SBUF copy (per-partition).
  - v natural [S, 512] with bias via a K=1 ones-row matmul; stored per
    (s-tile, head) as [v | 1] 65-column blocks for the fused rowsum trick.
  - scores^T tiles [128 keys, 1024 q] per (head, k-tile); both heads of a
    pair run concurrently on the PE via row-tiling (K=64 each,
    tile_position (0,0)/(64,0)). exp on ScalarE with scale=1/8 and
    per-partition bias ln(mask[k]) gives masked e^T in one pass.
  - context: [v|1]^T @ e^T accumulated over k-tiles -> [65, q] PSUM
    (row 64 = masked rowsum for free in the same stream).
  - PE-transpose back to [q, 65], divide by (rowsum + 1e-8) on VectorE,
    DMA out fp32.
"""

import numpy as np

import concourse.bacc as bacc
import concourse.mybir as mybir
import concourse.tile as tile
from concourse.bass_utils import run_bass_kernel_spmd
from concourse.masks import make_identity

B, S, D = 4, 2048, 1024
H, DK = 16, 64
HPC = 8            # heads per core
OC = HPC * DK      # 512 output cols per core
P = 128
NT = S // P        # 16 s/k tiles
NDC = D // P       # 8 D chunks
NPAIR = HPC // 2   # 4 head pairs
QC = 1024          # attention q-chunk
NQC = S // QC      # 2
F32 = mybir.dt.float32
F16 = mybir.dt.float16

ACT_EXP = mybir.ActivationFunctionType.Exp
ACT_LOG = mybir.ActivationFunctionType.Ln


def build_nc():
    nc = bacc.Bacc(None, target_bir_lowering=False)

    Qd = nc.dram_tensor("Q", [S, D], F16, kind="ExternalInput")
    Kd = nc.dram_tensor("K", [S, D], F16, kind="ExternalInput")
    Vd = nc.dram_tensor("V", [S, D], F16, kind="ExternalInput")
    Wqd = nc.dram_tensor("Wq", [D, OC], F16, kind="ExternalInput")
    Wkd = nc.dram_tensor("Wk", [D, OC], F16, kind="ExternalInput")
    Wvd = nc.dram_tensor("Wv", [D, OC], F16, kind="ExternalInput")
    bqd = nc.dram_tensor("bq", [OC], F32, kind="ExternalInput")
    bkd = nc.dram_tensor("bk", [OC], F32, kind="ExternalInput")
    bvd = nc.dram_tensor("bv", [OC], F16, kind="ExternalInput")
    maskd = nc.dram_tensor("mask", [S], F32, kind="ExternalInput")
    outd = nc.dram_tensor("out", [S, OC], F32, kind="ExternalOutput")

    with tile.TileContext(nc) as tc:
        with (
            tc.tile_pool(name="small", bufs=1) as small,
            tc.tile_pool(name="wpool", bufs=2) as wpool,
            tc.tile_pool(name="xt", bufs=2) as xtp,
            tc.tile_pool(name="qkt", bufs=1) as qkt,
            tc.tile_pool(name="v2p", bufs=1) as v2p,
            tc.tile_pool(name="et", bufs=3) as etp,
            tc.tile_pool(name="cs", bufs=4) as csp,
            tc.tile_pool(name="nrm", bufs=2) as nrmp,
            tc.tile_pool(name="outs", bufs=2) as outsp,
            tc.tile_pool(name="psS", bufs=2, space="PSUM") as psS,
            tc.tile_pool(name="psC", bufs=4, space="PSUM") as psC,
        ):
            # ---- constants / small tensors ----
            ident = small.tile([P, P], F32, tag="ident")
            make_identity(nc, ident[:])

            mask_sb = small.tile([P, NT], F32, tag="mask")
            nc.sync.dma_start(
                out=mask_sb[:], in_=maskd[:].rearrange("(t p) -> p t", p=P)
            )
            ln_sb = small.tile([P, NT], F32, tag="lnmask")
            nc.scalar.activation(ln_sb[:], mask_sb[:], ACT_LOG)

            bq_sb = small.tile([P, NPAIR], F32, tag="bq")
            nc.sync.dma_start(
                out=bq_sb[:], in_=bqd[:].rearrange("(r p) -> p r", p=P)
            )
            bk_sb = small.tile([P, NPAIR], F32, tag="bk")
            nc.sync.dma_start(
                out=bk_sb[:], in_=bkd[:].rearrange("(r p) -> p r", p=P)
            )
            bv_sb = small.tile([1, OC], F16, tag="bv")
            nc.sync.dma_start(
                out=bv_sb[:], in_=bvd[:].rearrange("(a c) -> a c", a=1)
            )
            ones_sb = small.tile([1, P], F16, tag="ones")
            nc.vector.memset(ones_sb[:], 1.0)

            # q^T / k^T, per head pair: [128 (2x64 dk), S]
            qt_sb = qkt.tile([P, NPAIR, S], F16, tag="qt")
            kt_sb = qkt.tile([P, NPAIR, S], F16, tag="kt")
            # v2: per (s-tile, head): [v (64) | ones (1)]
            v2_sb = v2p.tile([P, NT, HPC, DK + 1], F16, tag="v2")
            nc.vector.memset(v2_sb[:, :, :, DK : DK + 1], 1.0)

            def load_w(wd):
                w_sb = wpool.tile([P, NDC, OC], F16, tag="w")
                nc.sync.dma_start(
                    out=w_sb[:], in_=wd[:].rearrange("(c p) m -> p c m", p=P)
                )
                return w_sb

            def load_xt(xd):
                """X [S, D] fp16 in HBM -> X^T [128, NDC, S] via DMA-transpose."""
                xt_t = xtp.tile([P, NDC, S], F16, tag="xt")
                for dc in range(NDC):
                    nc.sync.dma_start(
                        out=xt_t[:, dc, :],
                        in_=xd[:, dc * P : (dc + 1) * P],
                        transpose=True,
                    )
                return xt_t

            # ---- phase 1a/1b: K then Q projections (transposed layout) ----
            for xd, wd, b_sb, dst in (
                (Kd, Wkd, bk_sb, kt_sb),
                (Qd, Wqd, bq_sb, qt_sb),
            ):
                w_sb = load_w(wd)
                xt_t = load_xt(xd)
                for ch in range(S // 512):
                    for pr in range(NPAIR):
                        ps_p = psC.tile([P, 512], F32, tag="c")
                        for dc in range(NDC):
                            nc.tensor.matmul(
                                ps_p[:],
                                w_sb[:, dc, pr * P : (pr + 1) * P],
                                xt_t[:, dc, ch * 512 : (ch + 1) * 512],
                                start=(dc == 0),
                                stop=(dc == NDC - 1),
                            )
                        nc.vector.tensor_scalar_add(
                            dst[:, pr, ch * 512 : (ch + 1) * 512],
                            ps_p[:],
                            b_sb[:, pr : pr + 1],
                        )

            # ---- phase 1c: V projection (natural layout) ----
            wv_sb = load_w(Wvd)
            vt_t = load_xt(Vd)
            for t in range(NT):
                ps_v = psC.tile([P, OC], F32, tag="c")
                for dc in range(NDC):
                    nc.tensor.matmul(
                        ps_v[:],
                        vt_t[:, dc, t * P : (t + 1) * P],
                        wv_sb[:, dc, :],
                        start=(dc == 0),
                        stop=False,
                    )
                # += ones^T @ bv  (broadcast bias over the 128 s rows)
                nc.tensor.matmul(
                    ps_v[:], ones_sb[:], bv_sb[:], start=False, stop=True
                )
                nc.vector.tensor_copy(
                    v2_sb[:, t, :, 0:DK],
                    ps_v[:].rearrange("p (h d) -> p h d", h=HPC),
                )

            # ---- phase 2: attention, per (head pair, q-chunk) ----
            for pr in range(NPAIR):
                for qc in range(NQC):
                    q0 = qc * QC
                    ctx = []
                    for i in range(4):  # h1a h1b h2a h2b
                        c_t = psC.tile([DK + 1, QC // 2], F32, tag="c")
                        ctx.append(c_t)
                    for t in range(NT):
                        ps_sc = []
                        e_t = []
                        for hi in range(2):
                            lo, hi_p = hi * DK, (hi + 1) * DK
                            ps_x = psS.tile([P, QC], F32, tag="s")
                            for j in range(QC // 512):
                                nc.tensor.matmul(
                                    ps_x[:, j * 512 : (j + 1) * 512],
                                    kt_sb[lo:hi_p, pr, t * P : (t + 1) * P],
                                    qt_sb[lo:hi_p, pr, q0 + j * 512 : q0 + (j + 1) * 512],
                                    start=True,
                                    stop=True,
                                    tile_position=(hi * DK, 0),
                                )
                            ps_sc.append(ps_x)
                        for hi in range(2):
                            e_x = etp.tile([P, QC], F16, tag="e")
                            nc.scalar.activation(
                                e_x[:],
                                ps_sc[hi][:],
                                ACT_EXP,
                                bias=ln_sb[:, t : t + 1],
                                scale=0.125,
                            )
                            e_t.append(e_x)
                        for hi in range(2):
                            h = 2 * pr + hi
                            for j in range(2):
                                nc.tensor.matmul(
                                    ctx[2 * hi + j][:],
                                    v2_sb[:, t, h, :],
                                    e_t[hi][:, j * 512 : (j + 1) * 512],
                                    start=(t == 0),
                                    stop=(t == NT - 1),
                                )

                    # normalize + emit: [65, 512] -> [q, 65] -> out
                    out_t = outsp.tile([P, QC // P, P], F32, tag="o")
                    for hi in range(2):
                        c_sb = []
                        for j in range(2):
                            s_t = csp.tile([DK + 1, QC // 2], F32, tag="cs")
                            nc.vector.tensor_copy(s_t[:], ctx[2 * hi + j][:])
                            c_sb.append(s_t)
                        ps_n = psS.tile([P, QC // P, DK + 1], F32, tag="s")
                        for qs in range(QC // P):
                            j, blk = qs // 4, qs % 4
                            nc.tensor.transpose(
                                ps_n[:, qs, :],
                                c_sb[j][:, blk * P : (blk + 1) * P],
                                ident[0 : DK + 1, 0 : DK + 1],
                            )
                        rt = nrmp.tile([P, QC // P], F32, tag="r")
                        nc.vector.tensor_scalar_add(
                            rt[:], ps_n[:, :, DK], 1e-8
                        )
                        nc.vector.reciprocal(rt[:], rt[:])
                        for qs in range(QC // P):
                            nc.vector.tensor_scalar_mul(
                                out_t[:, qs, hi * DK : (hi + 1) * DK],
                                ps_n[:, qs, 0:DK],
                                rt[:, qs : qs + 1],
                            )
                    nc.sync.dma_start(
                        out=outd[:].rearrange("(s p) c -> p s c", p=P)[
                            :, qc * (QC // P) : (qc + 1) * (QC // P),
                            pr * P : (pr + 1) * P,
                        ],
                        in_=out_t[:],
                    )

    nc.finalize()
    return nc


_NC_CACHE = None


def _get_nc():
    global _NC_CACHE
    if _NC_CACHE is None:
        _NC_CACHE = build_nc()
    return _NC_CACHE


def kernel(**inputs):
    nc = _get_nc()
    Q = np.asarray(inputs["Q"], dtype=np.float32)
    K = np.asarray(inputs["K"], dtype=np.float32)
    V = np.asarray(inputs["V"], dtype=np.float32)
    mask = np.asarray(inputs["mask"], dtype=np.float32)
    Wq = np.asarray(inputs["Wq"], dtype=np.float32)
    Wk = np.asarray(inputs["Wk"], dtype=np.float32)
    Wv = np.asarray(inputs["Wv"], dtype=np.float32)
    bq = np.asarray(inputs["bq"], dtype=np.float32)
    bk = np.asarray(inputs["bk"], dtype=np.float32)
    bv = np.asarray(inputs["bv"], dtype=np.float32)

    in_maps = []
    for c in range(8):
        b, hh = c // 2, c % 2
        sl = slice(hh * OC, (hh + 1) * OC)
        in_maps.append(
            {
                "Q": np.ascontiguousarray(Q[b].astype(np.float16)),
                "K": np.ascontiguousarray(K[b].astype(np.float16)),
                "V": np.ascontiguousarray(V[b].astype(np.float16)),
                "Wq": np.ascontiguousarray(Wq[:, sl].astype(np.float16)),
                "Wk": np.ascontiguousarray(Wk[:, sl].astype(np.float16)),
                "Wv": np.ascontiguousarray(Wv[:, sl].astype(np.float16)),
                "bq": np.ascontiguousarray(bq[sl]),
                "bk": np.ascontiguousarray(bk[sl]),
                "bv": np.ascontiguousarray(bv[sl].astype(np.float16)),
                "mask": np.ascontiguousarray(mask[b]),
            }
        )
    res = run_bass_kernel_spmd(nc, in_maps, list(range(8)))
    out = np.empty((B, S, H * DK), np.float32)
    for c in range(8):
        b, hh = c // 2, c % 2
        out[b][:, hh * OC : (hh + 1) * OC] = res.results[c]["out"]
    return out
